# revision 8
# baseline (speedup 1.0000x reference)
"""Trainium2 Bass kernel for the 5x5 circular-padded conv
   y = conv5x5_circular(x[16,64,384,768], w[64,64,5,5]) + b.

Shards the batch dim T=16 across 8 NeuronCores (2 images per core),
runs one SPMD Bass/Tile program, gathers the full output.

Per-core kernel (v5, fp16 4-tile): direct conv as 25 taps of
K=64(ci), M=64(co) fp16 matmuls keeping all four 64x64 PE quadrant
groups busy concurrently:
 - row split: image rows 0-191 in SBUF partitions 0-63 (PE row group
   0), rows 192-383 in partitions 64-127 (row group 64).
 - col split: even output rows accumulate in PSUM partitions 0-63
   (col group 0), odd rows in partitions 64-127 (col group 64).
Steady-state per-tile cadence equals the 384-col stream time (~163ns),
so the schedule only has to keep the PE fed:
 - PSUM sub-units of two 2-bank tiles (2 output rows x 2 W-halves per
   row-group half) ping-pong via bufs=2 pools; ScalarE and VectorE
   each drain one half (one 768-wide op), so the PE never waits.
 - full-row (768-wide) output DMAs, split across the scalar and sync
   HWDGE rings; input bands prefetched two bands ahead (bufs=4) so a
   band load is never stuck behind result stores in one FIFO.
Input is circularly padded and fp16-cast on the host -> [2,64,388,772].
"""

import numpy as np

import concourse.mybir as mybir
from concourse.tile import TileContext
from concourse import bacc
from concourse import bass_utils

F16 = mybir.dt.float16
F32 = mybir.dt.float32
AFT = mybir.ActivationFunctionType

N_CORES = 8
T, C, H, W = 16, 64, 384, 768
Hh = H // 2          # rows per PE row-group half
Hp, Wp = H + 4, W + 4
RB = 8               # output rows per half per band
SU = 2               # output rows per half per PSUM sub-unit

_cache = {}


def _build_conv(T_loc):
    nbands = Hh // RB
    nsub = RB // SU
    nc = bacc.Bacc("TRN2", target_bir_lowering=False, debug=False)
    xp = nc.dram_tensor("xp", [T_loc, C, Hp, Wp], F16, kind="ExternalInput").ap()
    wd = nc.dram_tensor("wd", [128, 25 * 64], F16, kind="ExternalInput").ap()
    bd = nc.dram_tensor("bd", [128, 1], F32, kind="ExternalInput").ap()
    bdf = nc.dram_tensor("bdf", [128, 2, 384], F32, kind="ExternalInput").ap()
    y = nc.dram_tensor("y", [T_loc, C, H, W], F32, kind="ExternalOutput").ap()

    with TileContext(nc) as tc:
        with (
            tc.tile_pool(name="const", bufs=1) as cpool,
            tc.tile_pool(name="xband", bufs=4) as xpool,
            tc.tile_pool(name="yrow", bufs=6) as ypool,
            tc.tile_pool(name="psum", bufs=2, space="PSUM") as ppool,
        ):
            wsb = cpool.tile([128, 25 * 64], F16)
            nc.sync.dma_start(out=wsb, in_=wd)
            bsb = cpool.tile([128, 1], F32)
            nc.sync.dma_start(out=bsb, in_=bd)
            bsf = cpool.tile([128, 2, 384], F32)
            nc.sync.dma_start(out=bsf, in_=bdf)

            def load_band(t, u):
                r0 = u * RB
                xb = xpool.tile([128, RB + 4, Wp], F16, tag="xb", name="xb")
                nc.sync.dma_start(out=xb[0:64], in_=xp[t, :, r0 : r0 + RB + 4, :])
                nc.sync.dma_start(
                    out=xb[64:128], in_=xp[t, :, Hh + r0 : Hh + r0 + RB + 4, :]
                )
                return xb

            seq = [(t, u) for t in range(T_loc) for u in range(nbands)]
            pending = [load_band(*seq[0]), load_band(*seq[1])]
            for idx, (t, u) in enumerate(seq):
                xb = pending.pop(0)
                if idx + 2 < len(seq):
                    pending.append(load_band(*seq[idx + 2]))
                r0 = u * RB
                for j in range(nsub):
                    # sub-unit: rows r0+2j (even, col grp 0) and r0+2j+1
                    # (odd, col grp 64) for each row-group half; each ps
                    # tile spans 2 PSUM banks (one per W-half).
                    ps = {}
                    for half in (0, 1):
                        ps[half] = ppool.tile(
                            [128, 2, 512], F32, tag=f"ps{half}", name=f"ps{half}"
                        )
                    for s in range(25):
                        dy, dx = divmod(s, 5)
                        st = (s == 0)
                        sp = (s == 24)
                        for wbi in (0, 1):
                            off = wbi * 384 + dx
                            for half in (0, 1):
                                pb = 64 * half
                                lhsT = wsb[pb : pb + 64, s * 64 : (s + 1) * 64]
                                pst = ps[half]
                                nc.tensor.matmul(
                                    pst[0:64, wbi, 0:384],
                                    lhsT,
                                    xb[pb : pb + 64, 2 * j + dy, off : off + 384],
                                    start=st,
                                    stop=sp,
                                    skip_group_check=True,
                                )
                                nc.tensor.matmul(
                                    pst[64:128, wbi, 0:384],
                                    lhsT,
                                    xb[pb : pb + 64, 2 * j + 1 + dy, off : off + 384],
                                    start=st,
                                    stop=sp,
                                    skip_group_check=True,
                                )
                    for half in (0, 1):
                        pst = ps[half]
                        tout = ypool.tile([128, 2, 384], F32, tag="t", name="t")
                        if half == 0:
                            nc.scalar.activation(
                                tout, pst[:, :, 0:384], AFT.Identity, bias=bsb
                            )
                            dma_eng = nc.scalar
                        else:
                            nc.vector.tensor_add(out=tout, in0=pst[:, :, 0:384], in1=bsf)
                            dma_eng = nc.sync
                        h0 = half * Hh + r0 + 2 * j
                        dma_eng.dma_start(out=y[t, :, h0, :], in_=tout[0:64])
                        dma_eng.dma_start(out=y[t, :, h0 + 1, :], in_=tout[64:128])
    nc.compile()
    return nc


def prepare_in_maps(x, w, b):
    """Host-side prep: fp16 cast + circular pad, weight/bias layout, per-core shards."""
    x = np.asarray(x, dtype=np.float32)
    w = np.asarray(w, dtype=np.float32)
    b = np.asarray(b, dtype=np.float32)
    T_loc = T // N_CORES
    x16 = x.astype(np.float16)
    xpad = np.pad(x16, ((0, 0), (0, 0), (2, 2), (2, 2)), mode="wrap")
    # wd[ci, s*64+co] = w[co, ci, dy, dx], s = dy*5+dx; duplicated across
    # partition halves for the two PE row groups.
    wt = w.transpose(1, 2, 3, 0).reshape(64, 25 * 64).astype(np.float16)
    wdm = np.ascontiguousarray(np.concatenate([wt, wt], axis=0))
    b2 = np.concatenate([b, b]).astype(np.float32)
    bdm = b2.reshape(128, 1).copy()
    bdf = np.ascontiguousarray(
        np.broadcast_to(b2[:, None, None], (128, 2, 384))
    )
    return [
        {
            "xp": np.ascontiguousarray(xpad[c * T_loc : (c + 1) * T_loc]),
            "wd": wdm,
            "bd": bdm,
            "bdf": bdf,
        }
        for c in range(N_CORES)
    ]


def kernel(x, w, b):
    assert np.asarray(x).shape == (T, C, H, W)
    T_loc = T // N_CORES
    if "nc" not in _cache:
        _cache["nc"] = _build_conv(T_loc)
    nc = _cache["nc"]
    in_maps = prepare_in_maps(x, w, b)
    res = bass_utils.run_bass_kernel_spmd(nc, in_maps, core_ids=list(range(N_CORES)))
    return np.concatenate([res.results[c]["y"] for c in range(N_CORES)], axis=0)


# revision 9
# speedup vs baseline: 1.0662x; 1.0662x over previous
"""Trainium2 Bass kernel for the 5x5 circular-padded conv
   y = conv5x5_circular(x[16,64,384,768], w[64,64,5,5]) + b.

Shards the batch dim T=16 across 8 NeuronCores (2 images per core),
runs one SPMD Bass/Tile program, gathers the full output.

Per-core kernel (v5, fp16 4-tile): direct conv as 25 taps of
K=64(ci), M=64(co) fp16 matmuls keeping all four 64x64 PE quadrant
groups busy concurrently:
 - row split: image rows 0-191 in SBUF partitions 0-63 (PE row group
   0), rows 192-383 in partitions 64-127 (row group 64).
 - col split: even output rows accumulate in PSUM partitions 0-63
   (col group 0), odd rows in partitions 64-127 (col group 64).
Steady-state per-tile cadence equals the 384-col stream time (~163ns),
so the schedule only has to keep the PE fed:
 - PSUM sub-units of two 2-bank tiles (2 output rows x 2 W-halves per
   row-group half) ping-pong via bufs=2 pools; ScalarE and VectorE
   each drain one half (one 768-wide op), so the PE never waits.
 - full-row (768-wide) output DMAs, split across the scalar and sync
   HWDGE rings; input bands prefetched two bands ahead (bufs=4) so a
   band load is never stuck behind result stores in one FIFO.
Input is circularly padded and fp16-cast on the host -> [2,64,388,772].
"""

import numpy as np

import concourse.mybir as mybir
from concourse.tile import TileContext
from concourse import bacc
from concourse import bass_utils

F16 = mybir.dt.float16
F32 = mybir.dt.float32
AFT = mybir.ActivationFunctionType

N_CORES = 8
T, C, H, W = 16, 64, 384, 768
Hh = H // 2          # rows per PE row-group half
Hp, Wp = H + 4, W + 4
RB = 8               # output rows per half per band
SU = 2               # output rows per half per PSUM sub-unit

_cache = {}


def _build_conv(T_loc):
    nbands = Hh // RB
    nsub = RB // SU
    nc = bacc.Bacc("TRN2", target_bir_lowering=False, debug=False)
    xp = nc.dram_tensor("xp", [T_loc, C, Hp, Wp], F16, kind="ExternalInput").ap()
    wd = nc.dram_tensor("wd", [128, 25 * 64], F16, kind="ExternalInput").ap()
    bd = nc.dram_tensor("bd", [128, 1], F32, kind="ExternalInput").ap()
    bdf = nc.dram_tensor("bdf", [128, 2, 384], F32, kind="ExternalInput").ap()
    y = nc.dram_tensor("y", [T_loc, C, H, W], F32, kind="ExternalOutput").ap()

    with TileContext(nc) as tc:
        with (
            tc.tile_pool(name="const", bufs=1) as cpool,
            tc.tile_pool(name="xband", bufs=4) as xpool,
            tc.tile_pool(name="yrow", bufs=6) as ypool,
            tc.tile_pool(name="psum", bufs=2, space="PSUM") as ppool,
        ):
            wsb = cpool.tile([128, 25 * 64], F16)
            nc.sync.dma_start(out=wsb, in_=wd)
            bsb = cpool.tile([128, 1], F32)
            nc.sync.dma_start(out=bsb, in_=bd)
            bsf = cpool.tile([128, 2, 384], F32)
            nc.sync.dma_start(out=bsf, in_=bdf)

            def load_band(t, u):
                # smaller chunks keep HBM read bursts short so the PE
                # instruction prefetcher is never starved for long
                r0 = u * RB
                xb = xpool.tile([128, RB + 4, Wp], F16, tag="xb", name="xb")
                for c0, c1 in ((0, 4), (4, 8), (8, RB + 4)):
                    nc.sync.dma_start(
                        out=xb[0:64, c0:c1], in_=xp[t, :, r0 + c0 : r0 + c1, :]
                    )
                    nc.sync.dma_start(
                        out=xb[64:128, c0:c1],
                        in_=xp[t, :, Hh + r0 + c0 : Hh + r0 + c1, :],
                    )
                return xb

            seq = [(t, u) for t in range(T_loc) for u in range(nbands)]
            pending = [load_band(*seq[0]), load_band(*seq[1])]
            for idx, (t, u) in enumerate(seq):
                xb = pending.pop(0)
                if idx + 2 < len(seq):
                    pending.append(load_band(*seq[idx + 2]))
                r0 = u * RB
                for j in range(nsub):
                    # sub-unit: rows r0+2j (even, col grp 0) and r0+2j+1
                    # (odd, col grp 64) for each row-group half; each ps
                    # tile spans 2 PSUM banks (one per W-half).
                    ps = {}
                    for half in (0, 1):
                        ps[half] = ppool.tile(
                            [128, 2, 512], F32, tag=f"ps{half}", name=f"ps{half}"
                        )
                    for s in range(25):
                        dy, dx = divmod(s, 5)
                        st = (s == 0)
                        sp = (s == 24)
                        for wbi in (0, 1):
                            off = wbi * 384 + dx
                            for half in (0, 1):
                                pb = 64 * half
                                lhsT = wsb[pb : pb + 64, s * 64 : (s + 1) * 64]
                                pst = ps[half]
                                nc.tensor.matmul(
                                    pst[0:64, wbi, 0:384],
                                    lhsT,
                                    xb[pb : pb + 64, 2 * j + dy, off : off + 384],
                                    start=st,
                                    stop=sp,
                                    skip_group_check=True,
                                )
                                nc.tensor.matmul(
                                    pst[64:128, wbi, 0:384],
                                    lhsT,
                                    xb[pb : pb + 64, 2 * j + 1 + dy, off : off + 384],
                                    start=st,
                                    stop=sp,
                                    skip_group_check=True,
                                )
                    for half in (0, 1):
                        pst = ps[half]
                        tout = ypool.tile([128, 2, 384], F32, tag="t", name="t")
                        if half == 0:
                            nc.scalar.activation(
                                tout, pst[:, :, 0:384], AFT.Identity, bias=bsb
                            )
                            dma_eng = nc.scalar
                        else:
                            nc.vector.tensor_add(out=tout, in0=pst[:, :, 0:384], in1=bsf)
                            dma_eng = nc.sync
                        h0 = half * Hh + r0 + 2 * j
                        dma_eng.dma_start(out=y[t, :, h0, :], in_=tout[0:64])
                        dma_eng.dma_start(out=y[t, :, h0 + 1, :], in_=tout[64:128])
    nc.compile()
    return nc


def prepare_in_maps(x, w, b):
    """Host-side prep: fp16 cast + circular pad, weight/bias layout, per-core shards."""
    x = np.asarray(x, dtype=np.float32)
    w = np.asarray(w, dtype=np.float32)
    b = np.asarray(b, dtype=np.float32)
    T_loc = T // N_CORES
    x16 = x.astype(np.float16)
    xpad = np.pad(x16, ((0, 0), (0, 0), (2, 2), (2, 2)), mode="wrap")
    # wd[ci, s*64+co] = w[co, ci, dy, dx], s = dy*5+dx; duplicated across
    # partition halves for the two PE row groups.
    wt = w.transpose(1, 2, 3, 0).reshape(64, 25 * 64).astype(np.float16)
    wdm = np.ascontiguousarray(np.concatenate([wt, wt], axis=0))
    b2 = np.concatenate([b, b]).astype(np.float32)
    bdm = b2.reshape(128, 1).copy()
    bdf = np.ascontiguousarray(
        np.broadcast_to(b2[:, None, None], (128, 2, 384))
    )
    return [
        {
            "xp": np.ascontiguousarray(xpad[c * T_loc : (c + 1) * T_loc]),
            "wd": wdm,
            "bd": bdm,
            "bdf": bdf,
        }
        for c in range(N_CORES)
    ]


def kernel(x, w, b):
    assert np.asarray(x).shape == (T, C, H, W)
    T_loc = T // N_CORES
    if "nc" not in _cache:
        _cache["nc"] = _build_conv(T_loc)
    nc = _cache["nc"]
    in_maps = prepare_in_maps(x, w, b)
    res = bass_utils.run_bass_kernel_spmd(nc, in_maps, core_ids=list(range(N_CORES)))
    return np.concatenate([res.results[c]["y"] for c in range(N_CORES)], axis=0)


# revision 11
# speedup vs baseline: 1.0869x; 1.0194x over previous
"""Trainium2 Bass kernel for the 5x5 circular-padded conv
   y = conv5x5_circular(x[16,64,384,768], w[64,64,5,5]) + b.

Shards the batch dim T=16 across 8 NeuronCores (2 images per core),
runs one SPMD Bass/Tile program, gathers the full output.

Per-core kernel (v5, fp16 4-tile): direct conv as 25 taps of
K=64(ci), M=64(co) fp16 matmuls keeping all four 64x64 PE quadrant
groups busy concurrently:
 - row split: image rows 0-191 in SBUF partitions 0-63 (PE row group
   0), rows 192-383 in partitions 64-127 (row group 64).
 - col split: even output rows accumulate in PSUM partitions 0-63
   (col group 0), odd rows in partitions 64-127 (col group 64).
Steady-state per-tile cadence equals the 384-col stream time (~163ns),
so the schedule only has to keep the PE fed:
 - PSUM sub-units of two 2-bank tiles (2 output rows x 2 W-halves per
   row-group half) ping-pong via bufs=2 pools; ScalarE and VectorE
   each drain one half (one 768-wide op), so the PE never waits.
 - full-row (768-wide) output DMAs, split across the scalar and sync
   HWDGE rings; input bands prefetched two bands ahead (bufs=4) so a
   band load is never stuck behind result stores in one FIFO.
Input is circularly padded and fp16-cast on the host -> [2,64,388,772].
"""

import numpy as np

import concourse.mybir as mybir
from concourse.tile import TileContext
from concourse import bacc
from concourse import bass_utils

F16 = mybir.dt.float16
F32 = mybir.dt.float32
AFT = mybir.ActivationFunctionType

N_CORES = 8
T, C, H, W = 16, 64, 384, 768
Hh = H // 2          # rows per PE row-group half
Hp, Wp = H + 4, W + 4
RB = 8               # output rows per half per band
SU = 2               # output rows per half per PSUM sub-unit

_cache = {}


def _build_conv(T_loc):
    nbands = Hh // RB
    nsub = RB // SU
    nc = bacc.Bacc("TRN2", target_bir_lowering=False, debug=False)
    xp = nc.dram_tensor("xp", [T_loc, C, Hp, Wp], F16, kind="ExternalInput").ap()
    wd = nc.dram_tensor("wd", [128, 25 * 64], F16, kind="ExternalInput").ap()
    bd = nc.dram_tensor("bd", [128, 1], F32, kind="ExternalInput").ap()
    bdf = nc.dram_tensor("bdf", [128, 2, 384], F32, kind="ExternalInput").ap()
    y = nc.dram_tensor("y", [T_loc, C, H, W], F32, kind="ExternalOutput").ap()

    with TileContext(nc) as tc:
        with (
            tc.tile_pool(name="const", bufs=1) as cpool,
            tc.tile_pool(name="xband", bufs=4) as xpool,
            tc.tile_pool(name="yrow", bufs=6) as ypool,
            tc.tile_pool(name="psum", bufs=2, space="PSUM") as ppool,
        ):
            wsb = cpool.tile([128, 25 * 64], F16)
            nc.sync.dma_start(out=wsb, in_=wd)
            bsb = cpool.tile([128, 1], F32)
            nc.sync.dma_start(out=bsb, in_=bd)
            bsf = cpool.tile([128, 2, 384], F32)
            nc.sync.dma_start(out=bsf, in_=bdf)

            # Band loads are split into 3-row chunks and paced: one chunk
            # pair is emitted after each sub-unit, two bands ahead, so HBM
            # read bursts stay short and the PE instruction prefetcher is
            # never starved (long bursts caused periodic ~2-4us PE fetch
            # stalls at 256-instruction page boundaries).
            CH = [(c0, min(c0 + 3, RB + 4)) for c0 in range(0, RB + 4, 3)]

            def emit_chunk(xb, t, u, c0, c1):
                r0 = u * RB
                nc.sync.dma_start(
                    out=xb[0:64, c0:c1], in_=xp[t, :, r0 + c0 : r0 + c1, :]
                )
                nc.sync.dma_start(
                    out=xb[64:128, c0:c1],
                    in_=xp[t, :, Hh + r0 + c0 : Hh + r0 + c1, :],
                )

            seq = [(t, u) for t in range(T_loc) for u in range(nbands)]
            xb_tiles = []
            for k in (0, 1):
                xbk = xpool.tile([128, RB + 4, Wp], F16, tag="xb", name="xb")
                for c0, c1 in CH:
                    emit_chunk(xbk, *seq[k], c0, c1)
                xb_tiles.append(xbk)
            for idx, (t, u) in enumerate(seq):
                xb = xb_tiles.pop(0)
                chunks_next = None
                if idx + 2 < len(seq):
                    xbn = xpool.tile([128, RB + 4, Wp], F16, tag="xb", name="xb")
                    xb_tiles.append(xbn)
                    chunks_next = (xbn, seq[idx + 2])
                r0 = u * RB
                for j in range(nsub):
                    # sub-unit: rows r0+2j (even, col grp 0) and r0+2j+1
                    # (odd, col grp 64) for each row-group half; each ps
                    # tile spans 2 PSUM banks (one per W-half).
                    ps = {}
                    for half in (0, 1):
                        ps[half] = ppool.tile(
                            [128, 2, 512], F32, tag=f"ps{half}", name=f"ps{half}"
                        )
                    for s in range(25):
                        dy, dx = divmod(s, 5)
                        st = (s == 0)
                        sp = (s == 24)
                        for wbi in (0, 1):
                            off = wbi * 384 + dx
                            for half in (0, 1):
                                pb = 64 * half
                                lhsT = wsb[pb : pb + 64, s * 64 : (s + 1) * 64]
                                pst = ps[half]
                                nc.tensor.matmul(
                                    pst[0:64, wbi, 0:384],
                                    lhsT,
                                    xb[pb : pb + 64, 2 * j + dy, off : off + 384],
                                    start=st,
                                    stop=sp,
                                    skip_group_check=True,
                                )
                                nc.tensor.matmul(
                                    pst[64:128, wbi, 0:384],
                                    lhsT,
                                    xb[pb : pb + 64, 2 * j + 1 + dy, off : off + 384],
                                    start=st,
                                    stop=sp,
                                    skip_group_check=True,
                                )
                    for half in (0, 1):
                        pst = ps[half]
                        tout = ypool.tile([128, 2, 384], F32, tag="t", name="t")
                        if half == 0:
                            nc.scalar.activation(
                                tout, pst[:, :, 0:384], AFT.Identity, bias=bsb
                            )
                            dma_eng = nc.scalar
                        else:
                            nc.vector.tensor_add(out=tout, in0=pst[:, :, 0:384], in1=bsf)
                            dma_eng = nc.sync
                        h0 = half * Hh + r0 + 2 * j
                        dma_eng.dma_start(out=y[t, :, h0, :], in_=tout[0:64])
                        dma_eng.dma_start(out=y[t, :, h0 + 1, :], in_=tout[64:128])
                    if chunks_next is not None and j < len(CH):
                        xbn, (tn, un) = chunks_next
                        emit_chunk(xbn, tn, un, *CH[j])
    nc.compile()
    return nc


def prepare_in_maps(x, w, b):
    """Host-side prep: fp16 cast + circular pad, weight/bias layout, per-core shards."""
    x = np.asarray(x, dtype=np.float32)
    w = np.asarray(w, dtype=np.float32)
    b = np.asarray(b, dtype=np.float32)
    T_loc = T // N_CORES
    x16 = x.astype(np.float16)
    xpad = np.pad(x16, ((0, 0), (0, 0), (2, 2), (2, 2)), mode="wrap")
    # wd[ci, s*64+co] = w[co, ci, dy, dx], s = dy*5+dx; duplicated across
    # partition halves for the two PE row groups.
    wt = w.transpose(1, 2, 3, 0).reshape(64, 25 * 64).astype(np.float16)
    wdm = np.ascontiguousarray(np.concatenate([wt, wt], axis=0))
    b2 = np.concatenate([b, b]).astype(np.float32)
    bdm = b2.reshape(128, 1).copy()
    bdf = np.ascontiguousarray(
        np.broadcast_to(b2[:, None, None], (128, 2, 384))
    )
    return [
        {
            "xp": np.ascontiguousarray(xpad[c * T_loc : (c + 1) * T_loc]),
            "wd": wdm,
            "bd": bdm,
            "bdf": bdf,
        }
        for c in range(N_CORES)
    ]


def kernel(x, w, b):
    assert np.asarray(x).shape == (T, C, H, W)
    T_loc = T // N_CORES
    if "nc" not in _cache:
        _cache["nc"] = _build_conv(T_loc)
    nc = _cache["nc"]
    in_maps = prepare_in_maps(x, w, b)
    res = bass_utils.run_bass_kernel_spmd(nc, in_maps, core_ids=list(range(N_CORES)))
    return np.concatenate([res.results[c]["y"] for c in range(N_CORES)], axis=0)
